# revision 1
# baseline (speedup 1.0000x reference)
"""AuxLossFreeMoE TRN2 kernel: 16-expert top-2 sigmoid-gated MoE + shared expert.

Strategy (8 NeuronCores, one SPMD Bass program, per-core data via inputs):
  - Routing (sigmoid gating + top-2 + weight normalization) runs on host with
    the exact jax CPU ops of the reference: the random centroids saturate the
    sigmoid, producing thousands of exact ties broken by expert index, so any
    approximate device sigmoid (ACT LUT) flips selections. Routing is 0.13% of
    total FLOPs.
  - Precision split (gate: rel_err < 2e-2 on the max): routed experts compute
    in bf16 (PE streams bf16 at the same 1 col/cycle as fp32r, but DMA and
    SBUF traffic halve); the shared expert computes in fp8e4m3 with DoubleRow
    matmuls (2 MACs/cell/cycle, K=256 per instruction) — its 10x output
    damping (shared_expert_ratio=0.1) absorbs the fp8 quantization noise.
    Pow2 pre-scales (wgs*2^7, wus*2^3, wds*2^7) keep fp8 operands out of the
    subnormal range; the inverses fold exactly into the Silu eviction scale
    and the final Copy-eviction scale. Measured rel_err 1.30e-2.
  - Expert-parallel FFN with compile-time load balancing: the slot structure
    (per-core piece capacities, e.g. 416/320/256/128) is chosen AFTER routing
    by a small exact solver (_solve_assignment) that covers each expert's
    token count with 8-per-position piece inventory, minimizing padded
    capacity. Hot experts split across cores with tokens dealt capacity-
    proportionally in owner-strided order (_place_and_deal), flattening the
    per-(core, owner) contribution maxima that set the AllToAll bucket size.
    Up/gate matmuls sweep only the real capacity r_j; down-projection and
    scatter run on 128-slot tiles (ceil(r_j/128)).
  - The host pre-gathers and pre-transposes each core's tokens (dispatch-side
    sharding), so the device does pure dense SwiGLU: up/gate with stationary
    weights, transpose-free down-projection (h as the stationary operand,
    emitting token-major output directly), rows scaled by combine weight on
    PSUM eviction, then one 128-row indirect scatter per slot-tile into the
    owner-bucketed bf16 send buffer.
  - One AllToAll (bf16) moves contributions to token-owner cores. The shared
    expert is emitted after the collective so its PE work overlaps the wire
    time. Owners indirect-gather their two contributions per token, add the
    shared output, and write the final [512, 2048] f32 slice; the host
    concatenates the 8 slices.
"""

import os
import numpy as np
import ml_dtypes

NP_BF16 = ml_dtypes.bfloat16
NP_F8 = ml_dtypes.float8_e4m3
SH_SG = 7   # wgs pre-scale exponent (folded out inside the Silu eviction)
SH_SU = 3   # wus pre-scale exponent (rides through h)
SH_SD = 7   # wds pre-scale exponent

B, S, H = 4, 1024, 2048
E = 16
TOPK = 2
I = 1024
ISH = 2048
RATIO = 0.1
EPS = 1e-9
T = B * S
NC = 8
P = 128
TOWN = T // NC  # 512 tokens owned per core
# candidate slot structures (per-core piece capacities), tried in order.
# Layout size of a piece = ceil(r/128)*128; ug matmuls only sweep r columns.
SLOT_CONFIGS = [
    (416, 320, 256, 128),
    (448, 320, 256, 128),
    (512, 384, 256, 128),
    (512, 512, 384, 256),
    (768, 768, 768, 768),
]
KC_H = H // P    # 16
KC2 = H // 256   # 8 paired contraction chunks for DoubleRow
M_I = I // P     # 8
M_ISH = ISH // P  # 16
IC2 = ISH // 256  # 8 paired contraction chunks for the shared down-proj
DUMMY_TOK = T  # extra zero row in x_pad
BIG = 10 ** 9

_COMPILED = {}
SKIP_PHASES = frozenset()  # debug: subsets of {'shared','routed','a2a','combine'}


def _enable_jax_cache():
    import jax
    try:
        cache_dir = os.environ.get("KERNEL_JAX_CACHE", "/tmp/jax_moe_cache")
        jax.config.update("jax_compilation_cache_dir", cache_dir)
        jax.config.update("jax_persistent_cache_min_compile_time_secs", 0.0)
    except Exception:
        pass


def _host_routing(x, centroids, gate_bias):
    """Bit-identical routing to the reference (jax CPU ops)."""
    import jax
    import jax.numpy as jnp
    cpu = jax.devices("cpu")[0]
    with jax.default_device(cpu):
        xj = jax.device_put(np.asarray(x), cpu)
        cj = jax.device_put(np.asarray(centroids), cpu)
        gj = jax.device_put(np.asarray(gate_bias), cpu)
        aff = jax.nn.sigmoid(jnp.einsum('bsh,eh->bse', xj, cj))
        biased = aff + gj
        _, top_idx = jax.lax.top_k(biased, TOPK)
        top_aff = jnp.take_along_axis(aff, top_idx, axis=-1)
        weights = top_aff / (top_aff.sum(-1, keepdims=True) + EPS)
    top_idx = np.asarray(top_idx).reshape(T, TOPK).astype(np.int64)
    weights = np.asarray(weights).reshape(T, TOPK).astype(np.float32)
    return top_idx, weights


def _solve_assignment(rs, counts, node_budget=2000000):
    """Assign each expert a multiset of slot-position pieces (8 available per
    position) covering its token count; minimize total pieces. Returns
    {expert: (x_0..x_{k-1})} or None."""
    import itertools
    k = len(rs)
    order = sorted(range(len(counts)), key=lambda e: -counts[e])
    cnts = [counts[e] for e in order]
    best = [None]
    nodes = [0]

    def combos_for(c):
        out = []
        maxp = [min(NC, -(-c // r)) for r in rs]
        for x in itertools.product(*[range(m + 1) for m in maxp]):
            tot = sum(xi * r for xi, r in zip(x, rs))
            if tot < c or sum(x) == 0:
                continue
            if all(not (x[j] > 0 and tot - rs[j] >= c) for j in range(k)):
                out.append((sum(x), x))
        out.sort()
        return [x for _, x in out]

    all_combos = [combos_for(c) if c > 0 else [] for c in cnts]
    inv = [NC] * k
    acc = {}

    def dfs(i, n_pieces):
        nodes[0] += 1
        if nodes[0] > node_budget:
            return
        if best[0] is not None and n_pieces + (len(cnts) - i) >= best[0][0]:
            return
        if i == len(cnts):
            best[0] = (n_pieces, {order[j]: acc[j] for j in acc})
            return
        if cnts[i] == 0:
            acc[i] = tuple([0] * k)
            dfs(i + 1, n_pieces)
            del acc[i]
            return
        for x in all_combos[i]:
            if all(inv[j] >= x[j] for j in range(k)):
                for j in range(k):
                    inv[j] -= x[j]
                acc[i] = x
                dfs(i + 1, n_pieces + sum(x))
                del acc[i]
                for j in range(k):
                    inv[j] += x[j]

    dfs(0, 0)
    return best[0][1] if best[0] else None


def _owner_vec(toks):
    v = np.zeros(NC, np.int64)
    for t in toks:
        v[t // TOWN] += 1
    return v


def _place_and_deal(rs, assign, lists, wvals):
    """Deal each expert's tokens across its pieces capacity-proportionally
    (owner-strided), then place pieces on cores to flatten per-(core, owner)
    contribution maxima. Returns cores[c] = [(expert, tokens, weights), ...]
    one entry per slot position (expert=-1 for dummy pieces)."""
    k = len(rs)
    # per-expert piece list: positions repeated x_ej times
    pieces = []  # (expert, position, tokens, weights)
    for e, x in assign.items():
        pos = [j for j in range(k) for _ in range(x[j])]
        if not pos:
            continue
        caps = [rs[j] for j in pos]
        fills = [0] * len(pos)
        buckets = [[] for _ in pos]
        n = len(lists[e])
        for t in range(n):
            i = min((i for i in range(len(pos)) if fills[i] < caps[i]),
                    key=lambda i: fills[i] / caps[i])
            buckets[i].append(t)
            fills[i] += 1
        for i, j in enumerate(pos):
            toks = [lists[e][t] for t in buckets[i]]
            ws = [wvals[e][t] for t in buckets[i]]
            pieces.append((e, j, toks, ws))

    # place pieces on cores, greedily flattening (core, owner) maxima
    by_pos = [[] for _ in range(k)]
    for pc in pieces:
        by_pos[pc[1]].append(pc)
    for j in range(k):
        by_pos[j].sort(key=lambda pc: -_owner_vec(pc[2]).max())
        while len(by_pos[j]) < NC:
            by_pos[j].append((-1, j, [], []))

    core_load = [np.zeros(NC, np.int64) for _ in range(NC)]
    cores = [[None] * k for _ in range(NC)]
    for j in range(k):
        used = [False] * NC
        for pc in by_pos[j]:
            ov = _owner_vec(pc[2])
            c = min((c for c in range(NC) if not used[c]),
                    key=lambda c: (core_load[c] + ov).max())
            used[c] = True
            cores[c][j] = (pc[0], pc[2], pc[3])
            core_load[c] += ov
    # local swap improvement
    def max_bucket():
        return max(v.max() for v in core_load)
    import itertools as it
    for _ in range(30):
        improved = False
        for j in range(k):
            for c1, c2 in it.combinations(range(NC), 2):
                p1, p2 = cores[c1][j], cores[c2][j]
                v1, v2 = _owner_vec(p1[1]), _owner_vec(p2[1])
                cur = max((core_load[c1]).max(), (core_load[c2]).max())
                new = max((core_load[c1] - v1 + v2).max(),
                          (core_load[c2] - v2 + v1).max())
                if new < cur:
                    cores[c1][j], cores[c2][j] = p2, p1
                    core_load[c1] += v2 - v1
                    core_load[c2] += v1 - v2
                    improved = True
        if not improved:
            break
    return cores


def _build_program():
    """Build the SPMD Bass program (same for all cores)."""
    import concourse.bass as bass
    import concourse.mybir as mybir
    import concourse.tile as tile
    from concourse import bacc
    from concourse.masks import make_identity

    dt = mybir.dt
    AF = mybir.ActivationFunctionType
    ALU = mybir.AluOpType

    SCAP = _build_program.SCAP
    RS = _build_program.RS  # per-position ug widths, e.g. (416, 320, 256, 128)
    NSLOT = len(RS)
    SIZES_L = [((r + P - 1) // P) * P for r in RS]  # layout sizes (128-aligned)
    CAP_L = sum(SIZES_L)
    N_TILES_L = CAP_L // P
    TILE_OFF = [sum(SIZES_L[:j]) // P for j in range(NSLOT)]
    SEND_ROWS = NC * SCAP

    nc = bacc.Bacc("TRN2", target_bir_lowering=False, num_devices=NC)

    f32, i32 = dt.float32, dt.int32
    bf16, f8 = dt.bfloat16, dt.float8e4
    DR = mybir.MatmulPerfMode.DoubleRow

    xg_in = nc.dram_tensor("xg_in", [KC_H, P, CAP_L], bf16, kind="ExternalInput")
    wslot = nc.dram_tensor("wslot", [N_TILES_L, P], f32, kind="ExternalInput")
    send_pos = nc.dram_tensor("send_pos", [N_TILES_L, P], i32, kind="ExternalInput")
    recv_idx = nc.dram_tensor("recv_idx", [2, TOWN // P, P], i32, kind="ExternalInput")
    wg_in = nc.dram_tensor("wg_in", [NSLOT, M_I, P, KC_H, P], bf16, kind="ExternalInput")
    wu_in = nc.dram_tensor("wu_in", [NSLOT, M_I, P, KC_H, P], bf16, kind="ExternalInput")
    wd_in = nc.dram_tensor("wd_in", [NSLOT, M_I, P, H], bf16, kind="ExternalInput")
    # shared-expert weights, fp8 DoubleRow layout
    wgs_in = nc.dram_tensor("wgs_in", [M_ISH, P, KC2, 2, P], f8, kind="ExternalInput")
    wus_in = nc.dram_tensor("wus_in", [M_ISH, P, KC2, 2, P], f8, kind="ExternalInput")
    wds_in = nc.dram_tensor("wds_in", [IC2, P, 2, H], f8, kind="ExternalInput")
    xT_own = nc.dram_tensor("xT_own", [P, KC2, 2, TOWN], f8, kind="ExternalInput")

    out_own = nc.dram_tensor("out_own", [TOWN, H], f32, kind="ExternalOutput")

    send_buf = nc.dram_tensor("send_buf", [SEND_ROWS, H], bf16)
    recv_buf = nc.dram_tensor("recv_buf", [SEND_ROWS, H], bf16)

    # piece -> (local tile offset, number of slot tiles, matmul blocks)
    piece_tiles = [s // P for s in SIZES_L]
    piece_tile_off = TILE_OFF
    piece_blocks = {}
    for j, r in enumerate(RS):
        blocks = []
        b0 = 0
        while b0 < r:
            bn = min(512, r - b0)
            blocks.append((b0, bn))
            b0 += bn
        piece_blocks[j] = blocks
    MAX_SL = max(SIZES_L)

    with tile.TileContext(nc) as tc:
        with (
            tc.tile_pool(name="const", bufs=1) as constp,
            tc.tile_pool(name="big", bufs=1) as bigp,
            tc.tile_pool(name="io", bufs=2) as iop,
        ):
            shared_tok = bigp.tile([P, TOWN // P, H], f32, name="shared_tok")
            n_hb = H // 512

            # ---------------- routed experts (bf16, solver-sized pieces) ----------------
            if "routed" not in SKIP_PHASES:
              with (
                  tc.tile_pool(name="rtbig", bufs=1) as rtbig,
                  tc.tile_pool(name="rtw", bufs=2) as rtw,
                  tc.tile_pool(name="rtwork", bufs=2) as work,
              ):
                  for p_i in range(len(RS)):
                      n_t = piece_tiles[p_i]
                      t_off = piece_tile_off[p_i]

                      up_ps = tc.tile_pool(name=f"upps{p_i}", bufs=1, space="PSUM")
                      psp = up_ps.__enter__()

                      # load pre-gathered, pre-transposed tokens for this piece
                      # (one tile per kc so the first matmul only waits on kc=0)
                      sz_p = RS[p_i]
                      xgT = []
                      for kc in range(KC_H):
                          t_kc = rtbig.tile([P, MAX_SL], bf16, name=f"xgT{kc}",
                                            tag=f"xgT{kc}")
                          nc.sync.dma_start(
                              t_kc[:, :sz_p],
                              xg_in[kc, :, t_off * P:t_off * P + sz_p])
                          xgT.append(t_kc)
                      wts = []
                      sidx = []
                      for st in range(n_t):
                          w_t = constp.tile([P, 1], f32, name=f"w_t{p_i}_{st}", tag=f"w_t{t_off + st}")
                          nc.sync.dma_start(w_t[:], wslot[t_off + st][:, None])
                          wts.append(w_t)
                          si_t = constp.tile([P, 1], i32, name=f"si_t{p_i}_{st}", tag=f"si_t{t_off + st}")
                          nc.sync.dma_start(si_t[:], send_pos[t_off + st][:, None])
                          sidx.append(si_t)

                      # up/gate projections -> h [i, slots] bf16
                      h = rtbig.tile([P, M_I, MAX_SL], bf16, name="h", tag="h")
                      if RS[p_i] < SIZES_L[p_i]:
                          # zero the ug-trimmed tail so the down-proj never
                          # consumes uninitialized SBUF
                          nc.vector.memset(h[:, :, RS[p_i]:SIZES_L[p_i]], 0.0)
                      for m in range(M_I):
                          wg_t = rtw.tile([P, KC_H, P], bf16, name="wg_t", tag="wg_t")
                          wu_t = rtw.tile([P, KC_H, P], bf16, name="wu_t", tag="wu_t")
                          if "wdma" not in SKIP_PHASES:
                              nc.sync.dma_start(wg_t[:], wg_in[p_i, m])
                              nc.sync.dma_start(wu_t[:], wu_in[p_i, m])
                          for (b0, bn) in piece_blocks[p_i]:
                              if "mm" in SKIP_PHASES:
                                  continue
                              psg2 = psp.tile([P, 512], f32, name="psg2", tag="psg", bufs=2)
                              psu2 = psp.tile([P, 512], f32, name="psu2", tag="psu", bufs=2)
                              for kc in range(KC_H):
                                  nc.tensor.matmul(psg2[:, :bn], wg_t[:, kc, :],
                                                   xgT[kc][:, b0:b0 + bn],
                                                   start=(kc == 0), stop=(kc == KC_H - 1))
                              for kc in range(KC_H):
                                  nc.tensor.matmul(psu2[:, :bn], wu_t[:, kc, :],
                                                   xgT[kc][:, b0:b0 + bn],
                                                   start=(kc == 0), stop=(kc == KC_H - 1))
                              sg2 = work.tile([P, 512], f32, name="sg2", tag="sg2")
                              nc.scalar.activation(sg2[:, :bn], psg2[:, :bn], AF.Silu)
                              nc.vector.tensor_mul(h[:, m, b0:b0 + bn], sg2[:, :bn], psu2[:, :bn])

                      # down projection, token-major out; scale; scatter to send_buf
                      up_ps.__exit__(None, None, None)
                      dn_ps = tc.tile_pool(name=f"dnps{p_i}", bufs=1, space="PSUM")
                      dpsp = dn_ps.__enter__()
                      y_tok = [rtbig.tile([P, H], bf16, name=f"y_tok{st}", tag=f"y_tok{st}")
                               for st in range(n_t)]
                      for hb in range(n_hb):
                          ps_d = [dpsp.tile([P, 512], f32, name=f"ps_d{st}", tag=f"ps_d{st}")
                                  for st in range(n_t)]
                          for ic in range(M_I):
                              wd_t = rtw.tile([P, 512], bf16, name="wd_t", tag="wd_t", bufs=4)
                              if "wdma" not in SKIP_PHASES:
                                  nc.sync.dma_start(wd_t[:], wd_in[p_i, ic][:, hb * 512:(hb + 1) * 512])
                              for st in range(n_t):
                                  nc.tensor.matmul(ps_d[st][:], h[:, ic, st * P:(st + 1) * P],
                                                   wd_t[:], start=(ic == 0), stop=(ic == M_I - 1))
                          for st in range(n_t):
                              nc.vector.tensor_scalar_mul(
                                  y_tok[st][:, hb * 512:(hb + 1) * 512],
                                  ps_d[st][:], wts[st][:, :1])
                      for st in range(n_t):
                          if "scatter" in SKIP_PHASES:
                              continue
                          nc.gpsimd.indirect_dma_start(
                              out=send_buf[:, :], in_=y_tok[st][:],
                              out_offset=bass.IndirectOffsetOnAxis(ap=sidx[st][:, :1], axis=0),
                              in_offset=None,
                              bounds_check=SEND_ROWS - 1,
                              oob_is_err=False)
                      dn_ps.__exit__(None, None, None)

            # ---------------- all-to-all combine ----------------
            if "a2a" not in SKIP_PHASES:
              nc.gpsimd.collective_compute(
                "AllToAll",
                mybir.AluOpType.bypass,
                replica_groups=[list(range(NC))],
                ins=[send_buf[:, :].opt()],
                outs=[recv_buf[:, :].opt()],
              )

            # ---------------- shared expert (own 512 tokens, fp8 DoubleRow) ----------------
            if "shared" in SKIP_PHASES:
                nc.vector.memset(shared_tok[:], 0.0)
            else:
              with (
                  tc.tile_pool(name="shbig", bufs=1) as shbig,
                  tc.tile_pool(name="shw", bufs=2) as shw,
                  tc.tile_pool(name="shps", bufs=1, space="PSUM") as psp,
              ):
                  xTo = shbig.tile([P, KC2, 2, TOWN], f8, name="xTo")
                  nc.sync.dma_start(xTo[:], xT_own[:])

                  # h (scaled by 2^SH_SU) in fp8, laid out for the DoubleRow down-proj
                  hs = shbig.tile([P, M_ISH, TOWN], f8, name="hs")
                  for m in range(M_ISH):
                      wgs_t = shw.tile([P, KC2, 2, P], f8, name="wgs_t", tag="wgs_t", bufs=4)
                      wus_t = shw.tile([P, KC2, 2, P], f8, name="wus_t", tag="wus_t", bufs=4)
                      nc.sync.dma_start(wgs_t[:], wgs_in[m])
                      nc.sync.dma_start(wus_t[:], wus_in[m])
                      psg = psp.tile([P, TOWN], f32, name="psg", tag="psg", bufs=2)
                      psu = psp.tile([P, TOWN], f32, name="psu", tag="psu", bufs=2)
                      for kc in range(KC2):
                          nc.tensor.matmul(psg[:], wgs_t[:, kc], xTo[:, kc],
                                           start=(kc == 0), stop=(kc == KC2 - 1),
                                           perf_mode=DR)
                      for kc in range(KC2):
                          nc.tensor.matmul(psu[:], wus_t[:, kc], xTo[:, kc],
                                           start=(kc == 0), stop=(kc == KC2 - 1),
                                           perf_mode=DR)
                      sg = shw.tile([P, TOWN], f32, name="sg", tag="sg")
                      nc.scalar.activation(sg[:], psg[:], AF.Silu, scale=2.0 ** -SH_SG)
                      nc.vector.tensor_mul(hs[:, m, :], sg[:], psu[:])

                  # shared down-projection (DoubleRow), output token-major directly
                  for hb in range(n_hb):
                      ps_sh = [psp.tile([P, 512], f32, name=f"ps_sh{tt}", tag=f"ps_sh{tt}")
                               for tt in range(TOWN // P)]
                      for ic in range(IC2):
                          wds_t = shw.tile([P, 2, 512], f8, name="wds_t", tag="wds_t", bufs=4)
                          nc.sync.dma_start(wds_t[:], wds_in[ic][:, :, hb * 512:(hb + 1) * 512])
                          for tt in range(TOWN // P):
                              nc.tensor.matmul(ps_sh[tt][:], hs[:, 2 * ic:2 * ic + 2, tt * P:(tt + 1) * P],
                                               wds_t[:], start=(ic == 0), stop=(ic == IC2 - 1),
                                               perf_mode=DR)
                      for tt in range(TOWN // P):
                          nc.scalar.activation(shared_tok[:, tt, hb * 512:(hb + 1) * 512],
                                               ps_sh[tt][:], AF.Copy,
                                               scale=RATIO * 2.0 ** -(SH_SU + SH_SD))


            with tc.tile_pool(name="cmb", bufs=2) as cmb:
                for tt in range(TOWN // P):
                    i1 = iop.tile([P, 1], i32, name="i1", tag="i1")
                    i2 = iop.tile([P, 1], i32, name="i2", tag="i2")
                    nc.sync.dma_start(i1[:], recv_idx[0, tt][:, None])
                    nc.sync.dma_start(i2[:], recv_idx[1, tt][:, None])
                    g1 = cmb.tile([P, H], bf16, name="g1", tag="g1")
                    g2 = cmb.tile([P, H], bf16, name="g2", tag="g2")
                    nc.gpsimd.indirect_dma_start(
                        out=g1[:], out_offset=None, in_=recv_buf[:, :],
                        in_offset=bass.IndirectOffsetOnAxis(ap=i1[:, :1], axis=0))
                    nc.gpsimd.indirect_dma_start(
                        out=g2[:], out_offset=None, in_=recv_buf[:, :],
                        in_offset=bass.IndirectOffsetOnAxis(ap=i2[:, :1], axis=0))
                    acc = cmb.tile([P, H], f32, name="acc", tag="acc")
                    nc.vector.tensor_add(acc[:], g1[:], g2[:])
                    # add + store in 512-col blocks so the early blocks (whose
                    # shared_tok slices finish first) overlap the shared tail
                    for hb in range(H // 512):
                        blk = slice(hb * 512, (hb + 1) * 512)
                        nc.vector.tensor_add(acc[:, blk], acc[:, blk],
                                             shared_tok[:, tt, blk])
                        nc.sync.dma_start(out_own[tt * P:(tt + 1) * P, blk],
                                          acc[:, blk])

    nc.finalize()
    return nc


def prepare_in_maps(x, centroids, gate_bias, wg_s, wu_s, wd_s, wg, wu, wd):
    x = np.ascontiguousarray(np.asarray(x, dtype=np.float32))
    wg = np.asarray(wg, dtype=np.float32)
    wu = np.asarray(wu, dtype=np.float32)
    wd = np.asarray(wd, dtype=np.float32)

    top_idx, weights = _host_routing(x, centroids, gate_bias)

    # expert token lists in token order
    lists = [[] for _ in range(E)]
    wvals = [[] for _ in range(E)]
    for t in range(T):
        for k in range(TOPK):
            e = int(top_idx[t, k])
            lists[e].append(t)
            wvals[e].append(weights[t, k])
    counts = [len(l) for l in lists]
    for rs in SLOT_CONFIGS:
        assign = _solve_assignment(rs, counts)
        if assign is not None:
            break
    else:
        raise RuntimeError(f"no slot config fits expert counts {counts}")
    cores = _place_and_deal(rs, assign, lists, wvals)

    sizes_l = [((r + P - 1) // P) * P for r in rs]
    cap_l = sum(sizes_l)
    n_tiles_l = cap_l // P
    piece_off = [sum(sizes_l[:j]) for j in range(len(rs))]

    # per-core slot tables
    tok_ids = np.full((NC, n_tiles_l, P), DUMMY_TOK, dtype=np.int32)
    wslot = np.zeros((NC, n_tiles_l, P), dtype=np.float32)
    piece_expert = np.full((NC, len(rs)), -1, dtype=np.int64)
    for c in range(NC):
        for pi, (e, toks, ws) in enumerate(cores[c]):
            piece_expert[c, pi] = e
            loc = piece_off[pi]
            pts = sorted(zip(toks, ws), key=lambda tw: (tw[0] // TOWN, tw[0]))
            assert len(pts) <= rs[pi]
            for j, (t, w) in enumerate(pts):
                tok_ids[c, (loc + j) // P, (loc + j) % P] = t
                wslot[c, (loc + j) // P, (loc + j) % P] = w

    # send positions / recv indices
    cnt_co = np.zeros((NC, NC), dtype=np.int64)
    contrib = [[] for _ in range(T)]  # (core, pos) per contribution
    for c in range(NC):
        for loc in range(cap_l):
            t = int(tok_ids[c, loc // P, loc % P])
            if t == DUMMY_TOK:
                continue
            o = t // TOWN
            pos = cnt_co[c, o]
            cnt_co[c, o] += 1
            contrib[t].append((c, int(pos)))
    SCAP = int(((cnt_co.max() + 15) // 16) * 16)
    # destination row = owner * SCAP + pos
    send_pos_arr = np.full((NC, n_tiles_l, P), BIG, dtype=np.int32)
    cnt_co2 = np.zeros((NC, NC), dtype=np.int64)
    for c in range(NC):
        for loc in range(cap_l):
            t = int(tok_ids[c, loc // P, loc % P])
            if t == DUMMY_TOK:
                continue
            o = t // TOWN
            pos = cnt_co2[c, o]
            cnt_co2[c, o] += 1
            send_pos_arr[c, loc // P, loc % P] = o * SCAP + pos

    recv_idx = np.zeros((NC, 2, TOWN // P, P), dtype=np.int32)
    for t in range(T):
        o = t // TOWN
        tl = t % TOWN
        assert len(contrib[t]) == 2, (t, contrib[t])
        for k, (c, pos) in enumerate(contrib[t]):
            recv_idx[o, k, tl // P, tl % P] = c * SCAP + pos

    # weight tensors, matmul-ready tiling (routed in bf16)
    def tile_up(w2d, mm):  # [H, mm*128] -> [mm, 128, KC_H, 128]
        return np.ascontiguousarray(
            w2d.reshape(KC_H, P, mm, P).transpose(2, 1, 0, 3).astype(NP_BF16))

    def tile_dn(w2d, mm):  # [mm*128, H] -> [mm, 128, H]
        return np.ascontiguousarray(w2d.reshape(mm, P, H).astype(NP_BF16))

    nslot = len(rs)
    wg_t = np.zeros((NC, nslot, M_I, P, KC_H, P), dtype=NP_BF16)
    wu_t = np.zeros((NC, nslot, M_I, P, KC_H, P), dtype=NP_BF16)
    wd_t = np.zeros((NC, nslot, M_I, P, H), dtype=NP_BF16)
    done = {}
    for c in range(NC):
        for pi, (e, toks, ws) in enumerate(cores[c]):
            if e < 0 or len(toks) == 0:
                continue
            if e not in done:
                done[e] = (tile_up(wg[e], M_I), tile_up(wu[e], M_I), tile_dn(wd[e], M_I))
            wg_t[c, pi], wu_t[c, pi], wd_t[c, pi] = done[e]

    # shared-expert weights in fp8 with pow2 pre-scales, DoubleRow pairing:
    # contraction index k = kc*256 + i*128 + p  ->  [.., p, kc, i, ..]
    def tile_up8(w2d, sc):  # [H, ISH] -> [M_ISH, P(p), KC2, 2(i), P(q)]
        w8 = (np.asarray(w2d, np.float32) * 2.0 ** sc).astype(NP_F8)
        return np.ascontiguousarray(
            w8.reshape(KC2, 2, P, M_ISH, P).transpose(3, 2, 0, 1, 4))

    def tile_dn8(w2d, sc):  # [ISH, H] -> [IC2, P(p), 2(i), H]
        w8 = (np.asarray(w2d, np.float32) * 2.0 ** sc).astype(NP_F8)
        return np.ascontiguousarray(
            w8.reshape(IC2, 2, P, H).transpose(0, 2, 1, 3))

    wgs_t = tile_up8(wg_s, SH_SG)
    wus_t = tile_up8(wu_s, SH_SU)
    wds_t = tile_dn8(wd_s, SH_SD)

    x_flat = x.reshape(T, H)
    x_bf = x_flat.astype(NP_BF16)
    x_pad = np.vstack([x_bf, np.zeros((1, H), NP_BF16)])
    x_f8 = x_flat.astype(NP_F8)

    in_maps = []
    for c in range(NC):
        # own tokens for the shared expert: [P(p), KC2, 2(i), TOWN]
        xo = np.ascontiguousarray(
            x_f8[c * TOWN:(c + 1) * TOWN].reshape(TOWN, KC2, 2, P).transpose(3, 1, 2, 0))
        xg_c = np.ascontiguousarray(
            x_pad[tok_ids[c].reshape(-1)].T.reshape(KC_H, P, cap_l))
        in_maps.append({
            "xg_in": xg_c,
            "wslot": wslot[c],
            "send_pos": send_pos_arr[c],
            "recv_idx": recv_idx[c],
            "wg_in": wg_t[c],
            "wu_in": wu_t[c],
            "wd_in": wd_t[c],
            "wgs_in": wgs_t,
            "wus_in": wus_t,
            "wds_in": wds_t,
            "xT_own": xo,
        })

    return in_maps, (tuple(rs), SCAP)


def get_program(cfg):
    rs, scap = cfg
    key = ("moe", cfg)
    if key not in _COMPILED:
        _build_program.SCAP = scap
        _build_program.RS = rs
        _COMPILED[key] = _build_program()
    return _COMPILED[key]


_RUNNER = {}


def _build_runner(nc, n_cores=NC):
    """Build a reusable PJRT executable for the finalized Bass program.
    Mirrors concourse.bass2jax.run_bass_via_pjrt but without output donation,
    so the jitted callable can be invoked repeatedly and its HLO is stable
    across processes (persistent-cache friendly)."""
    import jax
    import concourse.mybir as mybir
    from concourse import bass2jax as b2j
    from jax.experimental.shard_map import shard_map
    from jax.sharding import Mesh, PartitionSpec, NamedSharding

    b2j.install_neuronx_cc_hook()
    partition_name = nc.partition_id_tensor.name if nc.partition_id_tensor else None
    in_names, out_names, out_avals, zero_outs = [], [], [], []
    for alloc in nc.m.functions[0].allocations:
        if not isinstance(alloc, mybir.MemoryLocationSet):
            continue
        name = alloc.memorylocations[0].name
        if alloc.kind == "ExternalInput":
            if name != partition_name:
                in_names.append(name)
        elif alloc.kind == "ExternalOutput":
            shape = tuple(alloc.tensor_shape)
            dtype = mybir.dt.np(alloc.dtype)
            out_avals.append(jax.core.ShapedArray(shape, dtype))
            out_names.append(name)
            zero_outs.append(np.zeros(shape, dtype))
    n_params = len(in_names)
    all_in_names = in_names + out_names
    if partition_name is not None:
        all_in_names = all_in_names + [partition_name]

    def _body(*args):
        operands = list(args)
        if partition_name is not None:
            operands.append(b2j.partition_id_tensor())
        outs = b2j._bass_exec_p.bind(
            *operands,
            out_avals=tuple(out_avals),
            in_names=tuple(all_in_names),
            out_names=tuple(out_names),
            lowering_input_output_aliases=(),
            sim_require_finite=True,
            sim_require_nnan=True,
            nc=nc,
        )
        return tuple(outs)

    devices = jax.devices()[:n_cores]
    mesh = Mesh(np.asarray(devices), ("core",))
    spec = PartitionSpec("core")
    sharded = jax.jit(
        shard_map(_body, mesh=mesh, in_specs=(spec,) * (n_params + len(out_names)),
                  out_specs=(spec,) * len(out_names), check_rep=False),
        keep_unused=True,
    )
    sh = NamedSharding(mesh, spec)

    def run(in_maps):
        concat_in = [
            np.concatenate([np.asarray(in_maps[c][nm]) for c in range(n_cores)], axis=0)
            for nm in in_names
        ]
        concat_zeros = [np.zeros((n_cores * z.shape[0], *z.shape[1:]), z.dtype)
                        for z in zero_outs]
        dev_in = [jax.device_put(a, sh) for a in concat_in]
        dev_zero = [jax.device_put(a, sh) for a in concat_zeros]
        out = sharded(*dev_in, *dev_zero)
        jax.block_until_ready(out)
        return ({nm: np.asarray(out[i]) for i, nm in enumerate(out_names)},
                (sharded, dev_in, dev_zero))

    return run


def kernel(x, centroids, gate_bias, wg_s, wu_s, wd_s, wg, wu, wd):
    _enable_jax_cache()
    in_maps, scap = prepare_in_maps(x, centroids, gate_bias, wg_s, wu_s, wd_s, wg, wu, wd)
    nc = get_program(scap)
    key = ("run", scap)
    if key not in _RUNNER:
        _RUNNER[key] = _build_runner(nc)
    outs, _ = _RUNNER[key](in_maps)
    out = outs["out_own"].reshape(NC, TOWN, H)
    return np.ascontiguousarray(out.reshape(B, S, H))



# revision 10
# speedup vs baseline: 3.0393x; 3.0393x over previous
"""AuxLossFreeMoE TRN2 kernel: 16-expert top-2 sigmoid-gated MoE + shared expert.

Strategy (8 NeuronCores, one SPMD Bass program, per-core data via inputs):
  - Routing (sigmoid gating + top-2 + weight normalization) runs on host with
    the exact jax CPU ops of the reference: the random centroids saturate the
    sigmoid, producing thousands of exact ties broken by expert index, so any
    approximate device sigmoid (ACT LUT) flips selections. Routing is 0.13% of
    total FLOPs.
  - Precision split (gate: rel_err < 2e-2 on the max): routed experts compute
    in bf16 (the fp8 noise would be undamped there — measured decomposition:
    routed-bf16 contributes 3.8e-3, shared-fp8 1.2e-2 of the 1.30e-2 total);
    the shared expert computes in fp8e4m3 with DoubleRow matmuls (2
    MACs/cell/cycle, K=256 per instruction) — its 10x output damping
    (shared_expert_ratio=0.1) absorbs the fp8 quantization noise. Pow2
    pre-scales (wgs*2^7, wus*2^3, wds*2^7) keep fp8 operands out of the
    subnormal range; the inverses fold exactly into the Silu eviction scale
    and the final Copy-eviction scale.
  - Expert-parallel FFN with compile-time load balancing: the slot structure
    (per-core piece capacities, e.g. 416/320/256/128) is chosen AFTER routing
    by a small exact solver (_solve_assignment) that covers each expert's
    token count with 8-per-position piece inventory, minimizing padded
    capacity. Hot experts split across cores with tokens dealt capacity-
    proportionally in owner-strided order (_place_and_deal), flattening the
    per-(core, owner) contribution maxima that set the AllToAll bucket sizes.
  - The AllToAll is CHUNKED per piece group: each group has its own send
    buffer and collective, fired as soon as that group's scatters land, so
    the wire time overlaps the remaining routed compute and the shared
    expert; only the last chunk's tail is exposed.
  - The host pre-gathers and pre-transposes each core's tokens (dispatch-side
    sharding), so the device does pure dense SwiGLU: up/gate with stationary
    weights, transpose-free down-projection (h as the stationary operand,
    emitting token-major output directly), rows scaled by combine weight on
    PSUM eviction, then one 128-row indirect scatter per slot-tile into the
    owner-bucketed bf16 send buffer of the piece's group.
  - DMA issues are batched (weights grouped per (piece, m)/(piece, hb)/m/hb,
    slot tables and recv indices in single transposed loads) to keep the
    sync-queue issue rate off the critical path.
  - Owners indirect-gather their two contributions per token from recv_buf,
    add the shared output, and write the final [512, 2048] f32 slice; the
    host concatenates the 8 slices.
"""

import os
import numpy as np
import ml_dtypes

NP_BF16 = ml_dtypes.bfloat16
NP_F8 = ml_dtypes.float8_e4m3
SH_SG = 7   # wgs pre-scale exponent (folded out inside the Silu eviction)
SH_SU = 3   # wus pre-scale exponent (rides through h)
SH_SD = 7   # wds pre-scale exponent

B, S, H = 4, 1024, 2048
E = 16
TOPK = 2
I = 1024
ISH = 2048
RATIO = 0.1
EPS = 1e-9
T = B * S
NC = 8
P = 128
TOWN = T // NC  # 512 tokens owned per core
# candidate slot structures (per-core piece capacities), tried in order.
# Layout size of a piece = ceil(r/128)*128; ug matmuls only sweep r columns.
SLOT_CONFIGS = [
    (416, 320, 256, 128),
    (448, 320, 256, 128),
    (512, 384, 256, 128),
    (512, 512, 384, 256),
    (768, 768, 768, 768),
]
KC_H = H // P    # 16
KC2 = H // 256   # 8 paired contraction chunks for DoubleRow
M_I = I // P     # 8
M_ISH = ISH // P  # 16
IC2 = ISH // 256  # 8 paired contraction chunks for the shared down-proj
N_HB = H // 512   # 4 output column blocks
DUMMY_TOK = T  # extra zero row in x_pad

_COMPILED = {}
SKIP_PHASES = frozenset()  # debug: subsets of {'shared','routed','a2a','combine'}


def _a2a_groups(nslot, rs):
    """Per-piece AllToAll chunks, in piece processing order: the smallest
    piece runs first so its chunk starts the collective chain early, the rest
    run big-to-small so the last exposed chunk is as small as possible."""
    env = os.environ.get("KERNEL_PIECE_ORDER")
    if env:
        order = [int(v) for v in env.split(",")]
    else:
        # a mid-sized piece first (its collective starts the chain as early as
        # possible without a DMA-bound warmup), then descending, smallest last
        # (the only collective whose wire time is exposed at the tail)
        order = sorted(range(nslot), key=lambda j: -rs[j])
        if nslot >= 3:
            order = [order[-2]] + [j for j in order[:-2]] + [order[-1]]
    return tuple((p,) for p in order)


def _enable_jax_cache():
    import jax
    try:
        cache_dir = os.environ.get("KERNEL_JAX_CACHE", "/tmp/jax_moe_cache")
        jax.config.update("jax_compilation_cache_dir", cache_dir)
        jax.config.update("jax_persistent_cache_min_compile_time_secs", 0.0)
    except Exception:
        pass


def _host_routing(x, centroids, gate_bias):
    """Bit-identical routing to the reference (jax CPU ops)."""
    import jax
    import jax.numpy as jnp
    cpu = jax.devices("cpu")[0]
    with jax.default_device(cpu):
        xj = jax.device_put(np.asarray(x), cpu)
        cj = jax.device_put(np.asarray(centroids), cpu)
        gj = jax.device_put(np.asarray(gate_bias), cpu)
        aff = jax.nn.sigmoid(jnp.einsum('bsh,eh->bse', xj, cj))
        biased = aff + gj
        _, top_idx = jax.lax.top_k(biased, TOPK)
        top_aff = jnp.take_along_axis(aff, top_idx, axis=-1)
        weights = top_aff / (top_aff.sum(-1, keepdims=True) + EPS)
    top_idx = np.asarray(top_idx).reshape(T, TOPK).astype(np.int64)
    weights = np.asarray(weights).reshape(T, TOPK).astype(np.float32)
    return top_idx, weights


def _solve_assignment(rs, counts, node_budget=2000000):
    """Assign each expert a multiset of slot-position pieces (8 available per
    position) covering its token count; minimize total pieces. Returns
    {expert: (x_0..x_{k-1})} or None."""
    import itertools
    k = len(rs)
    order = sorted(range(len(counts)), key=lambda e: -counts[e])
    cnts = [counts[e] for e in order]
    best = [None]
    nodes = [0]

    def combos_for(c):
        out = []
        maxp = [min(NC, -(-c // r)) for r in rs]
        for x in itertools.product(*[range(m + 1) for m in maxp]):
            tot = sum(xi * r for xi, r in zip(x, rs))
            if tot < c or sum(x) == 0:
                continue
            if all(not (x[j] > 0 and tot - rs[j] >= c) for j in range(k)):
                out.append((sum(x), x))
        out.sort()
        return [x for _, x in out]

    all_combos = [combos_for(c) if c > 0 else [] for c in cnts]
    inv = [NC] * k
    acc = {}

    def dfs(i, n_pieces):
        nodes[0] += 1
        if nodes[0] > node_budget:
            return
        if best[0] is not None and n_pieces + (len(cnts) - i) >= best[0][0]:
            return
        if i == len(cnts):
            best[0] = (n_pieces, {order[j]: acc[j] for j in acc})
            return
        if cnts[i] == 0:
            acc[i] = tuple([0] * k)
            dfs(i + 1, n_pieces)
            del acc[i]
            return
        for x in all_combos[i]:
            if all(inv[j] >= x[j] for j in range(k)):
                for j in range(k):
                    inv[j] -= x[j]
                acc[i] = x
                dfs(i + 1, n_pieces + sum(x))
                del acc[i]
                for j in range(k):
                    inv[j] += x[j]

    dfs(0, 0)
    return best[0][1] if best[0] else None


def _owner_vec(toks):
    v = np.zeros(NC, np.int64)
    for t in toks:
        v[t // TOWN] += 1
    return v


def _place_and_deal(rs, assign, lists, wvals):
    """Deal each expert's tokens across its pieces capacity-proportionally
    (owner-strided), then place pieces on cores to flatten per-(core, owner)
    contribution maxima. Returns cores[c] = [(expert, tokens, weights), ...]
    one entry per slot position (expert=-1 for dummy pieces)."""
    k = len(rs)
    # per-expert piece list: positions repeated x_ej times
    pieces = []  # (expert, position, tokens, weights)
    for e, x in assign.items():
        pos = [j for j in range(k) for _ in range(x[j])]
        if not pos:
            continue
        caps = [rs[j] for j in pos]
        fills = [0] * len(pos)
        # owner-stratified dealing: spread each owner's tokens across the
        # expert's pieces proportionally to capacity, so the per-(piece,
        # owner) maxima that size the per-chunk AllToAll buckets stay flat
        fills_o = [[0] * len(pos) for _ in range(NC)]
        buckets = [[] for _ in pos]
        n = len(lists[e])
        for t in range(n):
            o = lists[e][t] // TOWN
            i = min((i for i in range(len(pos)) if fills[i] < caps[i]),
                    key=lambda i: ((fills_o[o][i] + 1) / caps[i], fills[i] / caps[i]))
            buckets[i].append(t)
            fills[i] += 1
            fills_o[o][i] += 1
        for i, j in enumerate(pos):
            toks = [lists[e][t] for t in buckets[i]]
            ws = [wvals[e][t] for t in buckets[i]]
            pieces.append((e, j, toks, ws))

    # place pieces on cores, greedily flattening (core, owner) maxima
    by_pos = [[] for _ in range(k)]
    for pc in pieces:
        by_pos[pc[1]].append(pc)
    for j in range(k):
        by_pos[j].sort(key=lambda pc: -_owner_vec(pc[2]).max())
        while len(by_pos[j]) < NC:
            by_pos[j].append((-1, j, [], []))

    core_load = [np.zeros(NC, np.int64) for _ in range(NC)]
    cores = [[None] * k for _ in range(NC)]
    for j in range(k):
        used = [False] * NC
        for pc in by_pos[j]:
            ov = _owner_vec(pc[2])
            c = min((c for c in range(NC) if not used[c]),
                    key=lambda c: (core_load[c] + ov).max())
            used[c] = True
            cores[c][j] = (pc[0], pc[2], pc[3])
            core_load[c] += ov
    # local swap improvement
    import itertools as it
    for _ in range(30):
        improved = False
        for j in range(k):
            for c1, c2 in it.combinations(range(NC), 2):
                p1, p2 = cores[c1][j], cores[c2][j]
                v1, v2 = _owner_vec(p1[1]), _owner_vec(p2[1])
                cur = max((core_load[c1]).max(), (core_load[c2]).max())
                new = max((core_load[c1] - v1 + v2).max(),
                          (core_load[c2] - v2 + v1).max())
                if new < cur:
                    cores[c1][j], cores[c2][j] = p2, p1
                    core_load[c1] += v2 - v1
                    core_load[c2] += v1 - v2
                    improved = True
        if not improved:
            break
    return cores


def _build_program():
    """Build the SPMD Bass program (same for all cores)."""
    import concourse.bass as bass
    import concourse.mybir as mybir
    import concourse.tile as tile
    from concourse import bacc

    dt = mybir.dt
    AF = mybir.ActivationFunctionType
    ALU = mybir.AluOpType

    RS = _build_program.RS        # per-position ug widths, e.g. (416, 320, 256, 128)
    SCAPS = _build_program.SCAPS  # per-a2a-group bucket capacity
    GROUPS = _build_program.GROUPS
    NSLOT = len(RS)
    SIZES_L = [((r + P - 1) // P) * P for r in RS]  # layout sizes (128-aligned)
    CAP_L = sum(SIZES_L)
    N_TILES_L = CAP_L // P
    TILE_OFF = [sum(SIZES_L[:j]) // P for j in range(NSLOT)]
    GROUP_ROWS = [NC * s for s in SCAPS]
    GROUP_BASE = [sum(GROUP_ROWS[:g]) for g in range(len(GROUPS))]
    R_TOT = sum(GROUP_ROWS)
    group_of_piece = {p: gi for gi, ps in enumerate(GROUPS) for p in ps}
    last_of_group = {ps[-1]: gi for gi, ps in enumerate(GROUPS)}

    nc = bacc.Bacc("TRN2", target_bir_lowering=False, num_devices=NC)

    f32, i32 = dt.float32, dt.int32
    bf16, f8 = dt.bfloat16, dt.float8e4
    DR = mybir.MatmulPerfMode.DoubleRow

    xg_in = nc.dram_tensor("xg_in", [P, KC_H, CAP_L], bf16, kind="ExternalInput")
    wslot = nc.dram_tensor("wslot", [P, N_TILES_L], f32, kind="ExternalInput")
    send_pos = nc.dram_tensor("send_pos", [P, N_TILES_L], i32, kind="ExternalInput")
    recv_idx = nc.dram_tensor("recv_idx", [P, 2 * (TOWN // P)], i32, kind="ExternalInput")
    wgu_in = nc.dram_tensor("wgu_in", [NSLOT, M_I, P, 2, KC_H, P], bf16, kind="ExternalInput")
    wd_in = nc.dram_tensor("wd_in", [NSLOT, N_HB, P, M_I, 512], bf16, kind="ExternalInput")
    # shared-expert weights, fp8 DoubleRow layout
    wsus_in = nc.dram_tensor("wsus_in", [M_ISH, P, 2, KC2, 2, P], f8, kind="ExternalInput")
    wds_in = nc.dram_tensor("wds_in", [N_HB, P, IC2, 2, 512], f8, kind="ExternalInput")
    xT_own = nc.dram_tensor("xT_own", [P, KC2, 2, TOWN], f8, kind="ExternalInput")

    out_own = nc.dram_tensor("out_own", [TOWN, H], f32, kind="ExternalOutput")

    send_bufs = [nc.dram_tensor(f"send_buf{g}", [GROUP_ROWS[g], H], bf16)
                 for g in range(len(GROUPS))]
    recv_buf = nc.dram_tensor("recv_buf", [R_TOT, H], bf16)

    # piece -> (local tile offset, number of slot tiles, matmul blocks)
    piece_tiles = [s // P for s in SIZES_L]
    piece_tile_off = TILE_OFF
    piece_blocks = {}
    for j, r in enumerate(RS):
        blocks = []
        b0 = 0
        while b0 < r:
            bn = min(512, r - b0)
            blocks.append((b0, bn))
            b0 += bn
        piece_blocks[j] = blocks
    MAX_SL = max(SIZES_L)

    N_ITERS = getattr(_build_program, "N_ITERS", 1)

    with tile.TileContext(nc) as tc:
        with (
            tc.tile_pool(name="const", bufs=1) as constp,
            tc.tile_pool(name="big", bufs=1) as bigp,
        ):
            shared_tok = bigp.tile([P, TOWN // P, H], f32, name="shared_tok")

            # bench mode (N_ITERS>1): repeat the whole body back-to-back in one
            # NEFF so per-iteration device time can be separated from the
            # multi-ms axon launch overhead.
            for _it in range(N_ITERS):
              # slot tables + recv indices: single batched loads
              wts_all = constp.tile([P, N_TILES_L], f32, name="wts_all", tag="wts_all")
              sidx_all = constp.tile([P, N_TILES_L], i32, name="sidx_all", tag="sidx_all")
              ridx_all = constp.tile([P, 2 * (TOWN // P)], i32, name="ridx_all", tag="ridx_all")
              nc.sync.dma_start(wts_all[:], wslot[:, :])
              nc.sync.dma_start(sidx_all[:], send_pos[:, :])
              nc.sync.dma_start(ridx_all[:], recv_idx[:, :])

              # ---------------- routed experts (bf16, solver-sized pieces) ----------------
              if "routed" not in SKIP_PHASES:
                with (
                    tc.tile_pool(name=f"rtbig{_it}", bufs=1) as rtbig,
                    tc.tile_pool(name=f"rtw{_it}", bufs=2) as rtw,
                    tc.tile_pool(name=f"rtwork{_it}", bufs=2) as work,
                ):
                    for p_i in [p for ps in GROUPS for p in ps]:
                        n_t = piece_tiles[p_i]
                        t_off = piece_tile_off[p_i]

                        up_ps = tc.tile_pool(name=f"upps{_it}_{p_i}", bufs=1, space="PSUM")
                        psp = up_ps.__enter__()

                        # pre-gathered, pre-transposed tokens for this piece.
                        # Two DMAs: kc 0-1 first so the first matmul starts early.
                        sz_p = RS[p_i]
                        xgT = rtbig.tile([P, KC_H, MAX_SL], bf16, name="xgT",
                                         tag="xgT", bufs=1)
                        nc.sync.dma_start(
                            xgT[:, 0:2, :sz_p],
                            xg_in[:, 0:2, t_off * P:t_off * P + sz_p])
                        nc.sync.dma_start(
                            xgT[:, 2:KC_H, :sz_p],
                            xg_in[:, 2:KC_H, t_off * P:t_off * P + sz_p])

                        # up/gate projections -> h [i, slots] bf16
                        h = rtbig.tile([P, M_I, MAX_SL], bf16, name="h", tag="h")
                        if RS[p_i] < SIZES_L[p_i]:
                            # zero the ug-trimmed tail so the down-proj never
                            # consumes uninitialized SBUF
                            nc.vector.memset(h[:, :, RS[p_i]:SIZES_L[p_i]], 0.0)
                        for m in range(M_I):
                            wgu_t = rtw.tile([P, 2, KC_H, P], bf16, name="wgu_t", tag="wgu_t", bufs=3)
                            if "wdma" not in SKIP_PHASES:
                                nc.sync.dma_start(wgu_t[:], wgu_in[p_i, m])
                            for (b0, bn) in piece_blocks[p_i]:
                                if "mm" in SKIP_PHASES:
                                    continue
                                psg2 = psp.tile([P, 512], f32, name="psg2", tag="psg", bufs=2)
                                psu2 = psp.tile([P, 512], f32, name="psu2", tag="psu", bufs=2)
                                for kc in range(KC_H):
                                    nc.tensor.matmul(psg2[:, :bn], wgu_t[:, 0, kc, :],
                                                     xgT[:, kc, b0:b0 + bn],
                                                     start=(kc == 0), stop=(kc == KC_H - 1))
                                for kc in range(KC_H):
                                    nc.tensor.matmul(psu2[:, :bn], wgu_t[:, 1, kc, :],
                                                     xgT[:, kc, b0:b0 + bn],
                                                     start=(kc == 0), stop=(kc == KC_H - 1))
                                sg2 = work.tile([P, 512], f32, name="sg2", tag="sg2")
                                nc.scalar.activation(sg2[:, :bn], psg2[:, :bn], AF.Silu)
                                nc.vector.tensor_mul(h[:, m, b0:b0 + bn], sg2[:, :bn], psu2[:, :bn])

                        # down projection, token-major out; scale; scatter to send_buf
                        up_ps.__exit__(None, None, None)
                        dn_ps = tc.tile_pool(name=f"dnps{_it}_{p_i}", bufs=1, space="PSUM")
                        dpsp = dn_ps.__enter__()
                        y_tok = [bigp.tile([P, H], bf16, name=f"y_tok{st}",
                                           tag=f"y_tok{st}", bufs=2)
                                 for st in range(n_t)]
                        for hb in range(N_HB):
                            ps_d = [dpsp.tile([P, 512], f32, name=f"ps_d{st}", tag=f"ps_d{st}")
                                    for st in range(n_t)]
                            wd_t = rtw.tile([P, M_I, 512], bf16, name="wd_t", tag="wd_t", bufs=3)
                            if "wdma" not in SKIP_PHASES:
                                nc.sync.dma_start(wd_t[:], wd_in[p_i, hb])
                            for ic in range(M_I):
                                for st in range(n_t):
                                    nc.tensor.matmul(ps_d[st][:], h[:, ic, st * P:(st + 1) * P],
                                                     wd_t[:, ic, :], start=(ic == 0), stop=(ic == M_I - 1))
                            for st in range(n_t):
                                nc.vector.tensor_scalar_mul(
                                    y_tok[st][:, hb * 512:(hb + 1) * 512],
                                    ps_d[st][:], wts_all[:, t_off + st:t_off + st + 1])
                        gi = group_of_piece[p_i]
                        for st in range(n_t):
                            if "scatter" in SKIP_PHASES:
                                continue
                            nc.gpsimd.indirect_dma_start(
                                out=send_bufs[gi][:, :], in_=y_tok[st][:],
                                out_offset=bass.IndirectOffsetOnAxis(
                                    ap=sidx_all[:, t_off + st:t_off + st + 1], axis=0),
                                in_offset=None,
                                bounds_check=GROUP_ROWS[gi] - 1,
                                oob_is_err=False)
                        dn_ps.__exit__(None, None, None)

                        # fire this group's AllToAll chunk as soon as its
                        # scatters land; wire time overlaps remaining compute
                        if p_i in last_of_group and "a2a" not in SKIP_PHASES:
                            g = last_of_group[p_i]
                            nc.gpsimd.collective_compute(
                                "AllToAll",
                                ALU.bypass,
                                replica_groups=[list(range(NC))],
                                ins=[send_bufs[g][:, :].opt()],
                                outs=[recv_buf[GROUP_BASE[g]:GROUP_BASE[g] + GROUP_ROWS[g], :].opt()],
                            )

              # ---------------- shared expert (own 512 tokens, fp8 DoubleRow) ----------------
              if "shared" in SKIP_PHASES:
                  nc.vector.memset(shared_tok[:], 0.0)
              else:
                with (
                    tc.tile_pool(name=f"shbig{_it}", bufs=1) as shbig,
                    tc.tile_pool(name=f"shw{_it}", bufs=2) as shw,
                    tc.tile_pool(name=f"shps{_it}", bufs=1, space="PSUM") as psp,
                ):
                    xTo = bigp.tile([P, KC2, 2, TOWN], f8, name="xTo", tag="xTo")
                    nc.sync.dma_start(xTo[:], xT_own[:])

                    # h (scaled by 2^SH_SU) in fp8, laid out for the DoubleRow down-proj
                    hs = bigp.tile([P, M_ISH, TOWN], f8, name="hs", tag="hs")
                    for m in range(M_ISH):
                        wsus_t = shw.tile([P, 2, KC2, 2, P], f8, name="wsus_t", tag="wsus_t")
                        nc.sync.dma_start(wsus_t[:], wsus_in[m])
                        psg = psp.tile([P, TOWN], f32, name="psg", tag="psg", bufs=2)
                        psu = psp.tile([P, TOWN], f32, name="psu", tag="psu", bufs=2)
                        for kc in range(KC2):
                            nc.tensor.matmul(psg[:], wsus_t[:, 0, kc], xTo[:, kc],
                                             start=(kc == 0), stop=(kc == KC2 - 1),
                                             perf_mode=DR)
                        for kc in range(KC2):
                            nc.tensor.matmul(psu[:], wsus_t[:, 1, kc], xTo[:, kc],
                                             start=(kc == 0), stop=(kc == KC2 - 1),
                                             perf_mode=DR)
                        sg = shw.tile([P, TOWN], f32, name="sg", tag="sg")
                        nc.scalar.activation(sg[:], psg[:], AF.Silu, scale=2.0 ** -SH_SG)
                        nc.vector.tensor_mul(hs[:, m, :], sg[:], psu[:])

                    # shared down-projection (DoubleRow), output token-major directly
                    for hb in range(N_HB):
                        ps_sh = [psp.tile([P, 512], f32, name=f"ps_sh{tt}", tag=f"ps_sh{tt}")
                                 for tt in range(TOWN // P)]
                        wds_t = shw.tile([P, IC2, 2, 512], f8, name="wds_t", tag="wds_t")
                        nc.sync.dma_start(wds_t[:], wds_in[hb])
                        for ic in range(IC2):
                            for tt in range(TOWN // P):
                                nc.tensor.matmul(ps_sh[tt][:], hs[:, 2 * ic:2 * ic + 2, tt * P:(tt + 1) * P],
                                                 wds_t[:, ic], start=(ic == 0), stop=(ic == IC2 - 1),
                                                 perf_mode=DR)
                        for tt in range(TOWN // P):
                            nc.scalar.activation(shared_tok[:, tt, hb * 512:(hb + 1) * 512],
                                                 ps_sh[tt][:], AF.Copy,
                                                 scale=RATIO * 2.0 ** -(SH_SU + SH_SD))

              # ---------------- combine (all on gpsimd/SP: the tile scheduler
              # otherwise hoists combine DVE ops ahead of shared-expert muls,
              # head-of-line blocking the DVE queue on the last collective) ----
              for tt in range(TOWN // P):
                  g1 = bigp.tile([P, H], bf16, name="g1", tag="g1", bufs=2)
                  nc.gpsimd.indirect_dma_start(
                      out=g1[:], out_offset=None, in_=recv_buf[:, :],
                      in_offset=bass.IndirectOffsetOnAxis(
                          ap=ridx_all[:, tt:tt + 1], axis=0))
                  # second contribution accumulated by the DMA ALU (CCE)
                  nc.gpsimd.indirect_dma_start(
                      out=g1[:], out_offset=None, in_=recv_buf[:, :],
                      in_offset=bass.IndirectOffsetOnAxis(
                          ap=ridx_all[:, TOWN // P + tt:TOWN // P + tt + 1], axis=0),
                      compute_op=ALU.add)
                  acc = bigp.tile([P, H], f32, name="acc", tag="acc", bufs=2)
                  nc.gpsimd.tensor_add(acc[:], g1[:], shared_tok[:, tt, :])
                  nc.sync.dma_start(out_own[tt * P:(tt + 1) * P, :], acc[:])

    nc.finalize()
    return nc


def prepare_in_maps(x, centroids, gate_bias, wg_s, wu_s, wd_s, wg, wu, wd):
    x = np.ascontiguousarray(np.asarray(x, dtype=np.float32))
    wg = np.asarray(wg, dtype=np.float32)
    wu = np.asarray(wu, dtype=np.float32)
    wd = np.asarray(wd, dtype=np.float32)

    top_idx, weights = _host_routing(x, centroids, gate_bias)

    # expert token lists in token order
    lists = [[] for _ in range(E)]
    wvals = [[] for _ in range(E)]
    for t in range(T):
        for k in range(TOPK):
            e = int(top_idx[t, k])
            lists[e].append(t)
            wvals[e].append(weights[t, k])
    counts = [len(l) for l in lists]
    for rs in SLOT_CONFIGS:
        assign = _solve_assignment(rs, counts)
        if assign is not None:
            break
    else:
        raise RuntimeError(f"no slot config fits expert counts {counts}")
    cores = _place_and_deal(rs, assign, lists, wvals)

    nslot = len(rs)
    groups = _a2a_groups(nslot, rs)
    group_of_piece = {p: gi for gi, ps in enumerate(groups) for p in ps}
    sizes_l = [((r + P - 1) // P) * P for r in rs]
    cap_l = sum(sizes_l)
    n_tiles_l = cap_l // P
    piece_off = [sum(sizes_l[:j]) for j in range(nslot)]
    piece_of_loc = np.zeros(cap_l, np.int64)
    for j in range(nslot):
        piece_of_loc[piece_off[j]:piece_off[j] + sizes_l[j]] = j

    # per-core slot tables
    tok_ids = np.full((NC, n_tiles_l, P), DUMMY_TOK, dtype=np.int32)
    wslot = np.zeros((NC, n_tiles_l, P), dtype=np.float32)
    piece_expert = np.full((NC, nslot), -1, dtype=np.int64)
    for c in range(NC):
        for pi, (e, toks, ws) in enumerate(cores[c]):
            piece_expert[c, pi] = e
            loc = piece_off[pi]
            pts = sorted(zip(toks, ws), key=lambda tw: (tw[0] // TOWN, tw[0]))
            assert len(pts) <= rs[pi]
            for j, (t, w) in enumerate(pts):
                tok_ids[c, (loc + j) // P, (loc + j) % P] = t
                wslot[c, (loc + j) // P, (loc + j) % P] = w

    # per-a2a-group (core, owner) bucket capacities; send positions; recv rows
    ng = len(groups)
    cnt_gco = np.zeros((ng, NC, NC), dtype=np.int64)
    contrib = [[] for _ in range(T)]  # (core, group, pos) per contribution
    for c in range(NC):
        for loc in range(cap_l):
            t = int(tok_ids[c, loc // P, loc % P])
            if t == DUMMY_TOK:
                continue
            g = group_of_piece[int(piece_of_loc[loc])]
            o = t // TOWN
            pos = cnt_gco[g, c, o]
            cnt_gco[g, c, o] += 1
            contrib[t].append((c, g, int(pos)))
    scaps = tuple(int(max(4, cnt_gco[g].max())) for g in range(ng))
    group_rows = [NC * s for s in scaps]
    group_base = [sum(group_rows[:g]) for g in range(ng)]

    # send row within group buffer = owner * SCAP_g + pos; dummies get the
    # first OOB row (dropped by bounds_check, sim-safe)
    send_pos_arr = np.zeros((NC, n_tiles_l, P), dtype=np.int32)
    cnt2 = np.zeros((ng, NC, NC), dtype=np.int64)
    for c in range(NC):
        for loc in range(cap_l):
            t = int(tok_ids[c, loc // P, loc % P])
            g = group_of_piece[int(piece_of_loc[loc])]
            if t == DUMMY_TOK:
                send_pos_arr[c, loc // P, loc % P] = group_rows[g]
                continue
            o = t // TOWN
            pos = cnt2[g, c, o]
            cnt2[g, c, o] += 1
            send_pos_arr[c, loc // P, loc % P] = o * scaps[g] + pos

    # recv row (global) = group_base + src_core * SCAP_g + pos
    recv_idx = np.zeros((NC, 2, TOWN // P, P), dtype=np.int32)
    for t in range(T):
        o = t // TOWN
        tl = t % TOWN
        assert len(contrib[t]) == 2, (t, contrib[t])
        for k, (c, g, pos) in enumerate(contrib[t]):
            recv_idx[o, k, tl // P, tl % P] = group_base[g] + c * scaps[g] + pos

    # weight tensors, matmul-ready tiling (routed in bf16)
    def tile_up(w2d, mm):  # [H, mm*128] -> [mm, 128, KC_H, 128]
        return np.ascontiguousarray(
            w2d.reshape(KC_H, P, mm, P).transpose(2, 1, 0, 3).astype(NP_BF16))

    def tile_dn(w2d, mm):  # [mm*128, H] -> [N_HB, 128, mm, 512]
        return np.ascontiguousarray(
            w2d.reshape(mm, P, N_HB, 512).transpose(2, 1, 0, 3).astype(NP_BF16))

    wgu_t = np.zeros((NC, nslot, M_I, P, 2, KC_H, P), dtype=NP_BF16)
    wd_t = np.zeros((NC, nslot, N_HB, P, M_I, 512), dtype=NP_BF16)
    done = {}
    for c in range(NC):
        for pi, (e, toks, ws) in enumerate(cores[c]):
            if e < 0 or len(toks) == 0:
                continue
            if e not in done:
                done[e] = (tile_up(wg[e], M_I), tile_up(wu[e], M_I), tile_dn(wd[e], M_I))
            wgu_t[c, pi, :, :, 0] = done[e][0]
            wgu_t[c, pi, :, :, 1] = done[e][1]
            wd_t[c, pi] = done[e][2]

    # shared-expert weights in fp8 with pow2 pre-scales, DoubleRow pairing:
    # contraction index k = kc*256 + i*128 + p  ->  [.., p, kc, i, ..]
    def tile_up8(w2d, sc):  # [H, ISH] -> [M_ISH, P(p), KC2, 2(i), P(q)]
        w8 = (np.asarray(w2d, np.float32) * 2.0 ** sc).astype(NP_F8)
        return np.ascontiguousarray(
            w8.reshape(KC2, 2, P, M_ISH, P).transpose(3, 2, 0, 1, 4))

    def tile_dn8(w2d, sc):  # [ISH, H] -> [N_HB, P(p), IC2, 2(i), 512]
        w8 = (np.asarray(w2d, np.float32) * 2.0 ** sc).astype(NP_F8)
        return np.ascontiguousarray(
            w8.reshape(IC2, 2, P, N_HB, 512).transpose(3, 2, 0, 1, 4))

    wsus_t = np.ascontiguousarray(
        np.stack([tile_up8(wg_s, SH_SG), tile_up8(wu_s, SH_SU)], axis=2))
    wds_t = tile_dn8(wd_s, SH_SD)

    x_flat = x.reshape(T, H)
    x_bf = x_flat.astype(NP_BF16)
    x_pad = np.vstack([x_bf, np.zeros((1, H), NP_BF16)])
    x_f8 = x_flat.astype(NP_F8)

    in_maps = []
    for c in range(NC):
        # own tokens for the shared expert: [P(p), KC2, 2(i), TOWN]
        xo = np.ascontiguousarray(
            x_f8[c * TOWN:(c + 1) * TOWN].reshape(TOWN, KC2, 2, P).transpose(3, 1, 2, 0))
        # gathered tokens, partition-major: [P, KC_H, cap_l]
        xg_c = np.ascontiguousarray(
            x_pad[tok_ids[c].reshape(-1)].reshape(cap_l, KC_H, P).transpose(2, 1, 0))
        # recv_idx transposed to [P, 2 * TOWN//P] (col = k*(TOWN//P) + tt)
        ridx_c = np.ascontiguousarray(
            recv_idx[c].reshape(2 * (TOWN // P), P).T)
        in_maps.append({
            "xg_in": xg_c,
            "wslot": np.ascontiguousarray(wslot[c].T),
            "send_pos": np.ascontiguousarray(send_pos_arr[c].T),
            "recv_idx": ridx_c,
            "wgu_in": wgu_t[c],
            "wd_in": wd_t[c],
            "wsus_in": wsus_t,
            "wds_in": wds_t,
            "xT_own": xo,
        })

    return in_maps, (tuple(rs), scaps, groups)


def get_program(cfg):
    rs, scaps, groups = cfg
    key = ("moe", cfg, os.environ.get("KERNEL_BENCH_ITERS", "1"))
    if key not in _COMPILED:
        _build_program.RS = rs
        _build_program.SCAPS = scaps
        _build_program.GROUPS = groups
        _build_program.N_ITERS = int(os.environ.get("KERNEL_BENCH_ITERS", "1"))
        _COMPILED[key] = _build_program()
    return _COMPILED[key]


_RUNNER = {}


def _build_runner(nc, n_cores=NC):
    """Build a reusable PJRT executable for the finalized Bass program.
    Mirrors concourse.bass2jax.run_bass_via_pjrt but without output donation,
    so the jitted callable can be invoked repeatedly and its HLO is stable
    across processes (persistent-cache friendly)."""
    import jax
    import concourse.mybir as mybir
    from concourse import bass2jax as b2j
    from jax.experimental.shard_map import shard_map
    from jax.sharding import Mesh, PartitionSpec, NamedSharding

    b2j.install_neuronx_cc_hook()
    partition_name = nc.partition_id_tensor.name if nc.partition_id_tensor else None
    in_names, out_names, out_avals, zero_outs = [], [], [], []
    for alloc in nc.m.functions[0].allocations:
        if not isinstance(alloc, mybir.MemoryLocationSet):
            continue
        name = alloc.memorylocations[0].name
        if alloc.kind == "ExternalInput":
            if name != partition_name:
                in_names.append(name)
        elif alloc.kind == "ExternalOutput":
            shape = tuple(alloc.tensor_shape)
            dtype = mybir.dt.np(alloc.dtype)
            out_avals.append(jax.core.ShapedArray(shape, dtype))
            out_names.append(name)
            zero_outs.append(np.zeros(shape, dtype))
    n_params = len(in_names)
    all_in_names = in_names + out_names
    if partition_name is not None:
        all_in_names = all_in_names + [partition_name]

    def _body(*args):
        operands = list(args)
        if partition_name is not None:
            operands.append(b2j.partition_id_tensor())
        outs = b2j._bass_exec_p.bind(
            *operands,
            out_avals=tuple(out_avals),
            in_names=tuple(all_in_names),
            out_names=tuple(out_names),
            lowering_input_output_aliases=(),
            sim_require_finite=True,
            sim_require_nnan=True,
            nc=nc,
        )
        return tuple(outs)

    devices = jax.devices()[:n_cores]
    mesh = Mesh(np.asarray(devices), ("core",))
    spec = PartitionSpec("core")
    sharded = jax.jit(
        shard_map(_body, mesh=mesh, in_specs=(spec,) * (n_params + len(out_names)),
                  out_specs=(spec,) * len(out_names), check_rep=False),
        keep_unused=True,
    )
    sh = NamedSharding(mesh, spec)

    def run(in_maps):
        concat_in = [
            np.concatenate([np.asarray(in_maps[c][nm]) for c in range(n_cores)], axis=0)
            for nm in in_names
        ]
        concat_zeros = [np.zeros((n_cores * z.shape[0], *z.shape[1:]), z.dtype)
                        for z in zero_outs]
        dev_in = [jax.device_put(a, sh) for a in concat_in]
        dev_zero = [jax.device_put(a, sh) for a in concat_zeros]
        out = sharded(*dev_in, *dev_zero)
        jax.block_until_ready(out)
        return ({nm: np.asarray(out[i]) for i, nm in enumerate(out_names)},
                (sharded, dev_in, dev_zero))

    return run


def kernel(x, centroids, gate_bias, wg_s, wu_s, wd_s, wg, wu, wd):
    _enable_jax_cache()
    in_maps, cfg = prepare_in_maps(x, centroids, gate_bias, wg_s, wu_s, wd_s, wg, wu, wd)
    nc = get_program(cfg)
    key = ("run", cfg)
    if key not in _RUNNER:
        _RUNNER[key] = _build_runner(nc)
    outs, _ = _RUNNER[key](in_maps)
    out = outs["out_own"].reshape(NC, TOWN, H)
    return np.ascontiguousarray(out.reshape(B, S, H))
